# revision 38
# baseline (speedup 1.0000x reference)
"""Trainium2 Bass kernel for CrossTrans block (dense_transformer).

Computation (per batch b):
  x   = xx[:, 288:384]                      # query stream  [96, N]
  q   = Wq'@x + qb ; k = Wk'@xx + kb ; v = Wv'@xx + vb
  attn= softmax(q_h^T k_h) per head ; av = v_h @ attn^T
  y   = relu(Wo'@(x + Wp'@relu(av_norm)) + ob')
BN scales folded into weights on host; p_bias folded into o_bias.

Sharding: 8 cores = 4 batches x 2 query-halves; k/v recomputed per half.

Scores: keys on partitions, queries free; 3-term split-bf16 matmul
  [k_hi; k_lo; k_hi]^T . [q_hi; q_hi; q_lo]  (K=48, fp32-grade logits)
with replicas at two 64-row strips so consecutive score matmuls
alternate stationary rows and LDWEIGHTS pipelines. Replica DMAs ride
the GpSimd+Sync queues, keeping the ACT queue free for exp.

The softmax denominator rides the AV matmul: each head's bf16 V tile
gets a 33rd all-ones column, so the AV psum block [33 rows] carries dn
in its last row -- no separate ones-matmul pass (saves ~52us of PE
streaming vs computing dn separately). Two heads share one psum bank
(blocks at rows 0/64 via tile_position), dn rows are copied out,
reciprocal'd, broadcast across 32 partitions on GpSimd, and one
scalar_tensor_tensor emits relu(av)*recip into 32-row slices of a
3-head-packed [96, 384] tile, so the Wp stage is 2 accumulating
matmuls per query chunk instead of 6. Output stage (Wp, Wo) is bf16.

Schedule: the attention is software-pipelined at head-PAIR granularity
with the previous pair's AV c-steps woven BETWEEN the score groups --
the PE queue is in-order, so without the weave the scores stall on the
2-buffer psum rotation (gated by exp on ACT) and the whole pipeline
ping-pongs. The attention SBUF pool is reserved before the conv pools
so e-tiles do not alias conv space (which serialized conv->attention).
The final pair's first head is drained early; tensor-queue drains at
two slot boundaries give the power governor brief PE-idle windows.

Measured: 225-229us vs 244-245us baseline back-to-back (~7%), rel
err 3.1e-3 (budget 2e-2). Run-to-run thermal variance is +-20%, so
only interleaved A/B comparisons are meaningful.
The wall is the HAM/power governor clamping the PE to ~1.2GHz under
sustained load: the e-matrix passes (scores 124k cols + AV 124k cols
through the PE, one exp pass through ACT) dominate at ~0.65ns/col.
"""

import numpy as np

NUM_HEADS = 6
KD = 16
DH = 32
B, C, Himg, Wimg = 4, 384, 48, 48
N = Himg * Wimg          # 2304
NH = N // 2              # 1152 queries per core
DIM_S = C // 4           # 96
NHKD = NUM_HEADS * KD    # 96
DHALL = NUM_HEADS * DH   # 192
NCORES = 8

NCH = 384                # query chunk (free dim of score matmuls)
NNC = NH // NCH          # 3 query chunks per core
MCH = 128                # key chunk (partition tile)
NMCH = N // MCH          # 18 key chunks
MG = 3                   # key chunks per exp group (3 psum banks)
NG = NMCH // MG          # 6 groups
KT = C // 128            # 3 contraction tiles over channels
VW = 200                 # padded width of augmented V tiles (6*33 = 198)


def build_nc():
    import concourse.bacc as bacc
    import concourse.mybir as mybir
    from concourse.tile import TileContext

    fp32 = mybir.dt.float32
    bf16 = mybir.dt.bfloat16
    AF = mybir.ActivationFunctionType
    OP = mybir.AluOpType

    nc = bacc.Bacc("TRN2", target_bir_lowering=False)

    xx_d = nc.dram_tensor("xx", [C, N], fp32, kind="ExternalInput")
    xh_d = nc.dram_tensor("xh", [DIM_S, NH], fp32, kind="ExternalInput")
    wk_d = nc.dram_tensor("wkT", [C, NHKD], fp32, kind="ExternalInput")
    wv_d = nc.dram_tensor("wvT", [C, VW], bf16, kind="ExternalInput")
    wq_d = nc.dram_tensor("wqT", [DIM_S, NHKD], fp32, kind="ExternalInput")
    wpp_d = nc.dram_tensor("wppT", [DHALL, DIM_S], bf16,
                           kind="ExternalInput")
    wo_d = nc.dram_tensor("woT", [DIM_S, C], bf16, kind="ExternalInput")
    kb_d = nc.dram_tensor("kb", [NHKD, 1], fp32, kind="ExternalInput")
    qb_d = nc.dram_tensor("qb", [NHKD, 1], fp32, kind="ExternalInput")
    vbb_d = nc.dram_tensor("vbb", [128, VW], fp32, kind="ExternalInput")
    ob_d = nc.dram_tensor("ob", [128, 3], fp32, kind="ExternalInput")
    y_d = nc.dram_tensor("y", [C, NH], fp32, kind="ExternalOutput")

    xx_t = xx_d[:, :].rearrange("(t p) n -> t p n", p=128)   # [3,128,N]
    wk_t = wk_d[:, :].rearrange("(t p) m -> t p m", p=128)
    wv_t = wv_d[:, :].rearrange("(t p) m -> t p m", p=128)

    with TileContext(nc) as tc:
        with tc.tile_pool(name="persist", bufs=1) as pp:
            # ---- small weights / constants ----
            # startup ordering: q-conv deps (wq, xh) first on the scalar
            # queue, k-conv deps (wk) first on sync, so convs start ~4us
            wq_sb = pp.tile([DIM_S, NHKD], fp32, tag="wq")
            nc.scalar.dma_start(out=wq_sb, in_=wq_d[:, :])
            xh_sb = pp.tile([DIM_S, NH], fp32, tag="xh")
            nc.scalar.dma_start(out=xh_sb, in_=xh_d[:, :])
            wk_sb = [pp.tile([128, NHKD], fp32, tag=f"wk{t}", name=f"wk{t}")
                     for t in range(KT)]
            wv_sb = [pp.tile([128, VW], bf16, tag=f"wv{t}", name=f"wv{t}")
                     for t in range(KT)]
            for t in range(KT):
                nc.sync.dma_start(out=wk_sb[t], in_=wk_t[t])
            kb_sb = pp.tile([NHKD, 1], fp32, tag="kb")
            nc.gpsimd.dma_start(out=kb_sb, in_=kb_d[:, :])
            qb_sb = pp.tile([NHKD, 1], fp32, tag="qb")
            nc.gpsimd.dma_start(out=qb_sb, in_=qb_d[:, :])
            wpp_sb = [pp.tile([DIM_S, DIM_S], bf16, tag=f"wpp{g}",
                              name=f"wpp{g}") for g in range(2)]
            for g in range(2):
                nc.scalar.dma_start(out=wpp_sb[g],
                                    in_=wpp_d[DIM_S * g:DIM_S * (g + 1), :])
            wo_sb = pp.tile([DIM_S, C], bf16, tag="wo")
            nc.scalar.dma_start(out=wo_sb, in_=wo_d[:, :])
            vbb_sb = pp.tile([128, VW], fp32, tag="vbb")
            nc.scalar.dma_start(out=vbb_sb, in_=vbb_d[:, :])
            ob_sb = pp.tile([128, 3], fp32, tag="ob")
            nc.gpsimd.dma_start(out=ob_sb, in_=ob_d[:, :])
            for t in range(KT):
                nc.scalar.dma_start(out=wv_sb[t], in_=wv_t[t])
            # preload the exp table set so the first real EXP is cheap
            warm = pp.tile([1, 1], fp32, tag="warm")
            nc.vector.memset(warm, 1.0)
            warm2 = pp.tile([1, 1], fp32, tag="warm2")
            nc.scalar.activation(out=warm2, in_=warm, func=AF.Exp)

            # persistent attention operands
            # krep rows: {0:khi 16:klo 32:khi | 64: same}  (two strip
            # sets so consecutive score matmuls alternate tile rows and
            # LDWEIGHTS pipelines with the stream)
            krep = [pp.tile([112, N], bf16, tag=f"krep{h}", name=f"krep{h}")
                    for h in range(NUM_HEADS)]
            qrep = [pp.tile([112, NH], bf16, tag=f"qrep{h}", name=f"qrep{h}")
                    for h in range(NUM_HEADS)]
            vT = [pp.tile([128, VW], bf16, tag=f"vt{c}", name=f"vt{c}")
                  for c in range(NMCH)]

            # attention SBUF reserved BEFORE the conv pools so the
            # e/rhs tiles do not alias conv space (a conv-drain barrier)
            asb_cm = tc.tile_pool(name="attn_sb", bufs=1)
            asb = asb_cm.__enter__()

            # ---- conv phase ----
            with tc.tile_pool(name="convxx", bufs=1) as cxp, \
                 tc.tile_pool(name="convst", bufs=1) as cip, \
                 tc.tile_pool(name="convps", bufs=2, space="PSUM") as cps:
                xx_sb = [cxp.tile([128, N], fp32, tag=f"xx{t}", name=f"xx{t}")
                         for t in range(KT)]
                # quarter-major order: the first conv chunk (needs all 3
                # t-tiles of columns [0,384)) is ready after ~1/4 of xx
                qn = N // 4
                xeng = [nc.sync, nc.gpsimd]
                for qtr in range(4):
                    for t in range(KT):
                        xeng[(qtr * KT + t) % 2].dma_start(
                            out=xx_sb[t][:, qtr * qn:(qtr + 1) * qn],
                            in_=xx_t[t][:, qtr * qn:(qtr + 1) * qn])
                xx_bf = [asb.tile([128, N], bf16, tag=f"xb{t}", name=f"xb{t}")
                         for t in range(KT)]

                # dense q = WqT.T @ xh + qb, split hi/lo  [96, NH]
                q_hi = cip.tile([NHKD, NH], bf16, tag="q_hi")
                q_lo = cip.tile([NHKD, NH], bf16, tag="q_lo")
                for j in range(NNC):
                    sl = slice(j * NCH, (j + 1) * NCH)
                    psq = cps.tile([NHKD, NCH], fp32, tag="psq")
                    nc.tensor.matmul(out=psq, lhsT=wq_sb, rhs=xh_sb[:, sl],
                                     start=True, stop=True)
                    nc.vector.tensor_scalar(
                        out=q_hi[:, sl], in0=psq, scalar1=qb_sb[:, 0:1],
                        scalar2=None, op0=OP.add)
                    nc.vector.scalar_tensor_tensor(
                        out=q_lo[:, sl], in0=psq, scalar=qb_sb[:, 0:1],
                        in1=q_hi[:, sl], op0=OP.add, op1=OP.subtract)

                # dense k = WkT.T @ xx + kb, split hi/lo  [96, N],
                # pipelined over two N-halves so replicas start early
                k_hi = cip.tile([NHKD, N], bf16, tag="k_hi")
                k_lo = cip.tile([NHKD, N], bf16, tag="k_lo")
                dmae = [nc.gpsimd, nc.sync, nc.scalar]
                di = 0
                # q replicas first (q is ready earliest)
                for h in range(NUM_HEADS):
                    hs = slice(KD * h, KD * (h + 1))
                    for bp in (0, 64):
                        for s0, src_t in ((bp, q_hi), (bp + 16, q_hi),
                                          (bp + 32, q_lo)):
                            dmae[di % 3].dma_start(
                                out=qrep[h][s0:s0 + 16, :], in_=src_t[hs, :])
                            di += 1
                for half in range(2):
                    hsl = slice(half * (N // 2), (half + 1) * (N // 2))
                    for j in range(3 * half, 3 * half + 3):
                        sl = slice(j * NCH, (j + 1) * NCH)
                        psk = cps.tile([NHKD, NCH], fp32, tag="psk")
                        for t in range(KT):
                            nc.tensor.matmul(
                                out=psk, lhsT=wk_sb[t], rhs=xx_sb[t][:, sl],
                                start=(t == 0), stop=(t == KT - 1))
                        nc.vector.tensor_scalar(
                            out=k_hi[:, sl], in0=psk, scalar1=kb_sb[:, 0:1],
                            scalar2=None, op0=OP.add)
                        nc.vector.scalar_tensor_tensor(
                            out=k_lo[:, sl], in0=psk, scalar=kb_sb[:, 0:1],
                            in1=k_hi[:, sl], op0=OP.add, op1=OP.subtract)
                    kdmae = ([nc.gpsimd, nc.sync, nc.scalar]
                             if half == 0 else [nc.gpsimd, nc.sync])
                    nk = len(kdmae)
                    for h in range(NUM_HEADS):
                        hs = slice(KD * h, KD * (h + 1))
                        for bp in (0, 64):
                            for s0, src_t in ((bp, k_hi), (bp + 16, k_lo),
                                              (bp + 32, k_hi)):
                                kdmae[di % nk].dma_start(
                                    out=krep[h][s0:s0 + 16, hsl],
                                    in_=src_t[hs, hsl])
                                di += 1

                # bf16 xx for the vT conv (off the critical path)
                for t in range(KT):
                    nc.vector.tensor_copy(out=xx_bf[t], in_=xx_sb[t])

                # vT chunks 0-11 up front; 12-17 are woven into the
                # first attention slot (they are consumed late in slot 1)
                for c in range(12):
                    psv = cps.tile([128, VW], fp32, tag="psv")
                    for t in range(KT):
                        nc.tensor.matmul(
                            out=psv,
                            lhsT=xx_bf[t][:, c * MCH:(c + 1) * MCH],
                            rhs=wv_sb[t],
                            start=(t == 0), stop=(t == KT - 1))
                    nc.vector.tensor_tensor(
                        out=vT[c], in0=psv, in1=vbb_sb, op=OP.add)

            # ---- attention, software-pipelined by head-slot ----
            with tc.tile_pool(name="attn_ps", bufs=1, space="PSUM") as aps:
                NSLOT = NNC * NUM_HEADS
                e_t = {}
                rhs_t = {}

                def emit_score_group(j, h, g):
                    nsl = slice(j * NCH, (j + 1) * NCH)
                    et = e_t[(j, h)]
                    pss = aps.tile([128, 3 * 512], fp32, tag="pss", bufs=2)
                    for s in range(MG):
                        c = MG * g + s
                        bp = 64 * (c % 2)
                        nc.tensor.matmul(
                            out=pss[:, 512 * s:512 * s + NCH],
                            lhsT=krep[h][bp:bp + 48,
                                         c * MCH:(c + 1) * MCH],
                            rhs=qrep[h][bp:bp + 48, nsl],
                            start=True, stop=True)
                    src = pss.rearrange("p (b n) -> p b n", n=512)
                    dst = et[:, g * MG * NCH:(g + 1) * MG * NCH]
                    nc.scalar.activation(
                        out=dst.rearrange("p (b n) -> p b n", n=NCH),
                        in_=src[:, :, 0:NCH], func=AF.Exp)

                def emit_av_steps(av, j, hA, hB, c0, c1):
                    # two heads share one psum bank: hA's av+dn block at
                    # rows [0,33), hB's at [64,97); alternating
                    # tile_position cols 0/64 keeps LDWEIGHTS pipelined.
                    for c in range(c0, c1):
                        for base, h in ((0, hA), (64, hB)):
                            nc.tensor.matmul(
                                out=av[base:base + 33, :],
                                lhsT=vT[c][:, 33 * h:33 * h + 33],
                                rhs=e_t[(j, h)][:, c * NCH:(c + 1) * NCH],
                                start=(c == 0), stop=(c == NMCH - 1),
                                skip_group_check=True,
                                tile_position=(0, base))

                def emit_norm_head(av, j, h, base):
                    if True:
                        dnr = asb.tile([1, NCH], fp32, tag="dnr", bufs=2)
                        nc.vector.tensor_copy(
                            out=dnr, in_=av[base + DH:base + DH + 1, :])
                        rcp = asb.tile([1, NCH], fp32, tag="rcp", bufs=2)
                        nc.vector.reciprocal_approx_fast(out=rcp, in_=dnr)
                        rcpb = asb.tile([DH, NCH], fp32, tag="rcpb", bufs=2)
                        nc.gpsimd.partition_broadcast(out_ap=rcpb[:, :],
                                                      in_ap=rcp[:, :])
                        grp, row = h // 3, DH * (h % 3)
                        if (j, grp) not in rhs_t:
                            rhs_t[(j, grp)] = asb.tile(
                                [DIM_S, NCH], bf16, tag="rhs", bufs=4,
                                name=f"rhs{j}_{grp}")
                        nc.vector.scalar_tensor_tensor(
                            out=rhs_t[(j, grp)][row:row + DH, :],
                            in0=av[base:base + DH, :], scalar=0.0,
                            in1=rcpb, op0=OP.max, op1=OP.mult)
                        del e_t[(j, h)]

                def emit_norm(av, j, hA, hB):
                    emit_norm_head(av, j, hA, 0)
                    emit_norm_head(av, j, hB, 64)

                def emit_vchunk(c):
                    psv = aps.tile([128, VW], fp32, tag="av", bufs=2)
                    for t in range(KT):
                        nc.tensor.matmul(
                            out=psv,
                            lhsT=xx_bf[t][:, c * MCH:(c + 1) * MCH],
                            rhs=wv_sb[t],
                            start=(t == 0), stop=(t == KT - 1))
                    nc.vector.tensor_tensor(
                        out=vT[c], in0=psv, in1=vbb_sb, op=OP.add)

                def emit_out(j):
                    nsl = slice(j * NCH, (j + 1) * NCH)
                    psp = aps.tile([DIM_S, NCH], fp32, tag="av", bufs=2)
                    for g in range(2):
                        nc.tensor.matmul(out=psp, lhsT=wpp_sb[g],
                                         rhs=rhs_t.pop((j, g)),
                                         start=(g == 0), stop=(g == 1))
                    xres = asb.tile([DIM_S, NCH], bf16, tag="xres", bufs=2)
                    nc.vector.tensor_tensor(
                        out=xres, in0=psp, in1=xh_sb[:, nsl], op=OP.add)
                    for g in range(3):
                        psy = aps.tile([128, NCH], fp32, tag="av", bufs=2)
                        nc.tensor.matmul(
                            out=psy, lhsT=wo_sb[:, 128 * g:128 * (g + 1)],
                            rhs=xres, start=True, stop=True)
                        ysb = asb.tile([128, NCH], fp32, tag="ysb", bufs=3)
                        nc.vector.tensor_scalar(
                            out=ysb, in0=psy, scalar1=ob_sb[:, g:g + 1],
                            scalar2=0.0, op0=OP.add, op1=OP.max)
                        nc.sync.dma_start(
                            out=y_d[128 * g:128 * (g + 1), nsl], in_=ysb)

                # Per slot: 12 score groups (2 heads x 6) with the
                # previous pair's 18 AV c-steps woven between them, so
                # the in-order PE queue always has ready work while the
                # ACT engine drains the exp groups.
                pairs = [(j, 2 * p, 2 * p + 1)
                         for j in range(NNC) for p in range(3)]
                NP = len(pairs)
                for slot in range(NP + 1):
                    prev = pairs[slot - 1] if slot >= 1 else None
                    av = (aps.tile([97, NCH], fp32, tag="av", bufs=2,
                                   name=f"av{slot}")
                          if prev and slot < NP else None)
                    if slot < NP:
                        j, hA, hB = pairs[slot]
                        e_t[(j, hA)] = asb.tile(
                            [128, NMCH * NCH], bf16, tag="e", bufs=5,
                            name=f"e{j}_{hA}")
                        e_t[(j, hB)] = asb.tile(
                            [128, NMCH * NCH], bf16, tag="e", bufs=5,
                            name=f"e{j}_{hB}")
                        for gi in range(2 * NG):
                            h, g = (hA, gi // 2) if gi % 2 == 0 \
                                else (hB, gi // 2)
                            emit_score_group(j, h, g)
                            if slot == 0 and gi % 2 == 1:
                                emit_vchunk(12 + gi // 2)
                            if slot == NP - 1 and gi == 1:
                                av_last = aps.tile([97, NCH], fp32,
                                                   tag="av", bufs=2,
                                                   name="av_last")
                            if slot == NP - 1 and gi % 2 == 1 and gi >= 3:
                                gB = gi // 2
                                for c in range(3 * (gB - 1), 3 * gB):
                                    nc.tensor.matmul(
                                        out=av_last[64:97, :],
                                        lhsT=vT[c][:, 33 * hB:33 * hB + 33],
                                        rhs=e_t[(j, hB)][:,
                                                         c * NCH:(c + 1) * NCH],
                                        start=(c == 0), stop=False,
                                        skip_group_check=True,
                                        tile_position=(0, 64))
                            if prev:
                                c0 = (gi * NMCH) // (2 * NG)
                                c1 = ((gi + 1) * NMCH) // (2 * NG)
                                emit_av_steps(av, *prev, c0, c1)
                        if slot == NP - 1:
                            # the final pair's first-head AV runs here so
                            # the flush slot only finishes the second head
                            for c in range(NMCH):
                                nc.tensor.matmul(
                                    out=av_last[0:33, :],
                                    lhsT=vT[c][:, 33 * hA:33 * hA + 33],
                                    rhs=e_t[(j, hA)][:,
                                                     c * NCH:(c + 1) * NCH],
                                    start=(c == 0), stop=(c == NMCH - 1),
                                    skip_group_check=True,
                                    tile_position=(0, 0))
                            emit_norm_head(av_last, j, hA, 0)
                    elif prev:
                        jf, hAf, hBf = prev
                        for c in range(NMCH - 3, NMCH):
                            nc.tensor.matmul(
                                out=av_last[64:97, :],
                                lhsT=vT[c][:, 33 * hBf:33 * hBf + 33],
                                rhs=e_t[(jf, hBf)][:,
                                                   c * NCH:(c + 1) * NCH],
                                start=(c == 0), stop=(c == NMCH - 1),
                                skip_group_check=True,
                                tile_position=(0, 64))
                        av = av_last
                    if prev:
                        if slot == NP:
                            emit_norm_head(av, prev[0], prev[2], 64)
                        else:
                            emit_norm(av, *prev)
                        if prev[2] == 5:
                            emit_out(prev[0])
                    if slot in (3, 6):
                        nc.tensor.drain()
            asb_cm.__exit__(None, None, None)
    return nc


def _install_trace_shim():
    # The agent image's antenv lacks axon_hooks; rebuild the NTFF hook
    # from the boot module so run_bass_kernel_spmd(trace=True) works.
    import sys
    import types
    try:
        import antenv.axon_hooks  # noqa: F401
        return
    except ImportError:
        pass
    try:
        from trn_agent_boot.trn_boot import _ntff_profile_via_ctypes
        hook = _ntff_profile_via_ctypes("/opt/axon/libaxon_pjrt.so")
        mod = types.ModuleType("antenv.axon_hooks")
        mod.get_axon_ntff_profile_hook = lambda: hook
        mod.set_axon_ntff_profile_hook = lambda h: None
        sys.modules["antenv.axon_hooks"] = mod
    except Exception:
        pass


def kernel(**inputs):
    import os
    from concourse.bass_utils import run_bass_kernel_spmd
    from ml_dtypes import bfloat16

    xx = np.asarray(inputs["xx"], dtype=np.float32)
    Wq = np.asarray(inputs["Wq"], dtype=np.float32)
    Wk = np.asarray(inputs["Wk"], dtype=np.float32)
    Wv = np.asarray(inputs["Wv"], dtype=np.float32)
    Wp = np.asarray(inputs["Wp"], dtype=np.float32)
    Wo = np.asarray(inputs["Wo"], dtype=np.float32)

    wqT = np.ascontiguousarray((inputs["q_scale"][:, None] * Wq).T)
    wkT = np.ascontiguousarray((inputs["k_scale"][:, None] * Wk).T)
    wvT = (inputs["v_scale"][:, None] * Wv).T.astype(np.float32)
    wpT = (inputs["p_scale"][:, None] * Wp).T.astype(np.float32)
    Wo2 = inputs["o_scale"][:, None] * Wo
    woT = np.ascontiguousarray(Wo2.T)
    ob2 = inputs["o_bias"] + Wo2 @ inputs["p_bias"]
    ob = np.ascontiguousarray(ob2.reshape(3, 128).T)   # [128, 3]
    kb = np.ascontiguousarray(inputs["k_bias"][:, None])
    qb = np.ascontiguousarray(inputs["q_bias"][:, None])

    # augmented V weights: head h at cols [33h, 33h+32), ones col 33h+32
    wv_aug = np.zeros((C, VW), dtype=np.float32)
    vbb_aug = np.zeros((128, VW), dtype=np.float32)
    wpp = np.zeros((DHALL, DIM_S), dtype=np.float32)
    for h in range(NUM_HEADS):
        wv_aug[:, 33 * h:33 * h + DH] = wvT[:, DH * h:DH * (h + 1)]
        vbb_aug[:, 33 * h:33 * h + DH] = \
            inputs["v_bias"][None, DH * h:DH * (h + 1)]
        vbb_aug[:, 33 * h + DH] = 1.0
        wpp[DH * h:DH * (h + 1), :] = wpT[DH * h:DH * (h + 1), :]

    xx_flat = xx.reshape(B, C, N)
    shared = dict(wkT=wkT.astype(np.float32),
                  wvT=wv_aug.astype(bfloat16),
                  wqT=wqT.astype(np.float32),
                  wppT=np.ascontiguousarray(wpp).astype(bfloat16),
                  woT=woT.astype(bfloat16),
                  kb=kb.astype(np.float32), qb=qb.astype(np.float32),
                  vbb=vbb_aug.astype(np.float32), ob=ob.astype(np.float32))

    in_maps = []
    for core in range(NCORES):
        b, half = core // 2, core % 2
        xxb = np.ascontiguousarray(xx_flat[b])
        xh = np.ascontiguousarray(
            xx_flat[b][3 * DIM_S:, half * NH:(half + 1) * NH])
        in_maps.append(dict(xx=xxb, xh=xh, **shared))

    nc = build_nc()
    if not nc.is_finalized():
        nc.finalize()
    trace = bool(int(os.environ.get("KERNEL_TRACE", "0")))
    if trace:
        _install_trace_shim()
    res = run_bass_kernel_spmd(nc, in_maps, list(range(NCORES)),
                               trace=trace)
    if trace:
        kernel.last_result = res

    out = np.empty((B, C, N), dtype=np.float32)
    for core in range(NCORES):
        b, half = core // 2, core % 2
        out[b][:, half * NH:(half + 1) * NH] = res.results[core]["y"]
    return out.reshape(B, C, Himg, Wimg)
